# revision 26
# baseline (speedup 1.0000x reference)
"""Multi-head attention (multiquery K/V) Bass kernel for 8 trn2 NeuronCores.

Sharding: 8 cores = 2 batches x 4 query-row quarters. Each core computes the
full multiquery K/V projections for its batch (cheap, dk=64) and runs
attention + output projection for its 512 query rows over all 16 heads.
Output is a pure concatenation across cores -- no collectives.

Device data flow (everything "transposed" so matmuls contract on partitions):
  xS (host pre-tiled, bf16): contiguous per-partition slabs, split across
     the sync/scalar/gpsimd DMA queues (one queue tops out ~120GB/s)
  QT [dout, s]  = x @ w_q.T transposed (bf16 inputs -> fp32 psum -> f32r)
  K2T [128, t]  = K.T stacked twice (rows 0:64 and 64:128) so even/odd heads
                  run as concurrent row-tiled matmul pairs
  Vp [t, 65]    = V with a ones column (column 64 accumulates the softmax
                  denominator during the attn@V matmul)
  phase B per 4-head pass: scores_T[t,s] -> exp on ACT (the ~147us kernel
  floor; scale folded in) -> attn@V, software-pipelined with a tapered
  depth-5 schedule. The previous pass's normalize (save attn values to
  SBUF, approx-reciprocal of the denominator row, rank-1 matmul broadcast
  into the freed psum rows, multiply) is interleaved into the next pass's
  loop so ACT never idles.
  phase C: y = sum_g8 oTP[:,g8] @ woP[:,g8] (head pairs packed to K=128,
  bf16) with groups 0..5 emitted during the last normalize chain.
"""

import sys

import numpy as np

if "/opt/trn_rl_repo" not in sys.path:
    sys.path.insert(0, "/opt/trn_rl_repo")

B, S, D = 2, 2048, 1024
H, DK = 16, 64
P = 128
NCORES, GPB = 8, 4
SPB = S // GPB  # 512 query rows per core
KC = D // P  # 8 contraction subtiles over d_model
NT = S // P  # 16 key/t blocks
NSB = SPB // P  # 4 s blocks
NDF = D // 512  # 2 output column halves


USE_F32R = True


def build_bass(scale: float):
    import concourse.bacc as bacc
    import concourse.mybir as mybir
    import concourse.tile as tile
    from concourse.bass import ts

    fp32 = mybir.dt.float32
    bf16 = mybir.dt.bfloat16
    # matmul-operand dtype: float32r streams at 1 cycle/row (vs 4 for
    # float32) and is bit-identical fp32 in memory
    mdt = mybir.dt.float32r if USE_F32R else fp32
    Act = mybir.ActivationFunctionType

    nc = bacc.Bacc(None, target_bir_lowering=False)
    # host-pre-tiled layouts: every DMA below is a contiguous slab per
    # partition (no software descriptor generation on gpsimd)
    xS = nc.dram_tensor("xS", [P, 4 * KC, 512], bf16, kind="ExternalInput")
    cst = nc.dram_tensor("cst", [P, P], mdt, kind="ExternalInput")
    wqH = nc.dram_tensor("wqH", [P, KC, D], bf16, kind="ExternalInput")
    wkkH = nc.dram_tensor("wkkH", [P, KC, P], bf16, kind="ExternalInput")
    wvH = nc.dram_tensor("wvH", [P, KC, DK], bf16, kind="ExternalInput")
    woP = nc.dram_tensor("woP", [P, KC, D], bf16, kind="ExternalInput")
    y = nc.dram_tensor("y", [SPB, D], fp32, kind="ExternalOutput")

    with tile.TileContext(nc) as tc:
        with tc.tile_pool(name="persist", bufs=1) as pers:
            K2T = pers.tile([P, S], mdt, name="K2T")
            Vp = pers.tile([P, NT, DK + 1], mdt, name="Vp")
            QT = pers.tile([P, KC, SPB], mdt, name="QT")
            oTP = pers.tile([P, KC, SPB], bf16, name="oTP")
            cst_sb = pers.tile([P, P], mdt, name="cst")
            # cst cols 0:64 = identity(64) (rows 0:64), cols 64:128 = ones
            nc.sync.dma_start(cst_sb[:], cst[:])
            ident = cst_sb[0:DK, 0:DK]
            ones = cst_sb[:, DK:DK + DK]

            # ---------------- Phase A: projections ----------------
            with (
                tc.tile_pool(name="pa", bufs=1) as pa,
                tc.tile_pool(name="paps", bufs=1, space="PSUM") as paps,
            ):
                wkk_sb = pa.tile([P, KC, P], bf16, name="wkk")
                wv_sb = pa.tile([P, KC, DK], bf16, name="wv")
                nc.sync.dma_start(wkk_sb[:], wkkH[:])
                nc.gpsimd.dma_start(wv_sb[:], wvH[:])
                wq_sb = pa.tile([P, KC, D], bf16, name="wq")

                for c in range(4):
                    xc = pa.tile([P, KC, 512], bf16, name="xc", tag="xc", bufs=2)
                    # x blocks spread across the three DMA queues (a single
                    # queue tops out well below the per-core HBM bandwidth);
                    # xc1/xc3 ride the gpsimd queue, which carries nothing
                    # else after startup
                    o = c * KC
                    if c == 0:
                        nc.sync.dma_start(xc[:, 0:3, :], xS[:, o:o + 3, :])
                        nc.scalar.dma_start(xc[:, 3:6, :], xS[:, o + 3:o + 6, :])
                        nc.gpsimd.dma_start(xc[:, 6:8, :], xS[:, o + 6:o + 8, :])
                        # ones column of Vp (strided; keep off the xc0 path)
                        nc.gpsimd.dma_start(Vp[:, :, DK], cst[:, DK:DK + NT])
                        # w_q after xc0 so the first K/V matmuls and the Q
                        # k-loop aren't stuck behind the 4MB transfer
                        for k in range(KC):
                            (nc.sync if k % 2 == 0 else nc.scalar).dma_start(
                                wq_sb[:, k, :], wqH[:, k, :]
                            )
                    elif c == 1 or c == 3:
                        nc.gpsimd.dma_start(xc[:], xS[:, o:o + KC, :])
                    else:
                        nc.sync.dma_start(xc[:, 0:4, :], xS[:, o:o + 4, :])
                        nc.scalar.dma_start(xc[:, 4:8, :], xS[:, o + 4:o + 8, :])

                    k2ps = paps.tile([P, 512], fp32, name="k2ps", tag="k2ps", bufs=2)
                    for k in range(KC):
                        nc.tensor.matmul(
                            k2ps[:], wkk_sb[:, k, :], xc[:, k, :],
                            start=(k == 0), stop=(k == KC - 1),
                        )
                    nc.vector.tensor_copy(K2T[:, ts(c, 512)], k2ps[:])

                    # V.T then PE-transpose into V' natural [t, dv]
                    vps = paps.tile([DK, 512], fp32, name="vps", tag="vps", bufs=2)
                    for k in range(KC):
                        nc.tensor.matmul(
                            vps[:], wv_sb[:, k, :], xc[:, k, :],
                            start=(k == 0), stop=(k == KC - 1),
                        )
                    vsb = pa.tile([DK, 512], mdt, name="vsb", tag="vsb", bufs=2)
                    nc.vector.tensor_copy(vsb[:], vps[:])
                    for q in range(4):
                        trps = paps.tile([P, DK], mdt, name="trps", tag="trps", bufs=2)
                        nc.tensor.transpose(trps[:], vsb[:, ts(q, P)], ident)
                        nc.vector.tensor_copy(Vp[:, 4 * c + q, 0:DK], trps[:])

                    if c == 0:
                        for m in range(KC):
                            qps = paps.tile([P, 512], fp32, name="qps", tag="qps", bufs=2)
                            for k in range(KC):
                                nc.tensor.matmul(
                                    qps[:], wq_sb[:, k, ts(m, P)], xc[:, k, :],
                                    start=(k == 0), stop=(k == KC - 1),
                                )
                            nc.vector.tensor_copy(QT[:, m, :], qps[:])

            # ---------------- Phase B: attention ----------------
            with tc.tile_pool(name="pb", bufs=1) as pb:
                wo_sb = pb.tile([P, KC, D], bf16, name="wo")
                nc.sync.dma_start(wo_sb[:], woP[:])

                from concourse.dve_ops import (
                    RECIP_APPROX_FAST_CONSTS as _RC,
                    RECIPROCAL_APPROX_FAST as _RF,
                )

                with tc.tile_pool(name="pbps", bufs=1, space="PSUM") as pbps:
                    # attnV emission schedule: deep (5) at pass start so the
                    # previous pass's normalize has time to release the outps
                    # banks, shallow at pass end so the PE tail is short and
                    # the next pass's scores/exp restart quickly
                    SCHED = {t: [] for t in range(NT)}
                    for k in range(9):
                        SCHED[k + 5].append(k)
                    SCHED[14] += [9, 10]
                    SCHED[15] += [11, 12, 13]
                    TAIL_ATTNV = [14, 15]

                    def emit_ocp_one(outps_p, g, on_act=False):
                        # save attn values to SBUF; frees the outps bank to
                        # receive the bc broadcast. On the last pass ACT is
                        # idle, so the copies go there.
                        ocp = pb.tile([DK, 2, 512], fp32,
                                      name=f"ocp{g}", tag=f"ocp{g}", bufs=2)
                        if on_act:
                            nc.scalar.copy(ocp[:], outps_p[g][0:DK, :, :])
                        else:
                            nc.vector.tensor_copy(
                                ocp[:], outps_p[g][0:DK, :, :]
                            )
                        return [ocp]

                    def emit_dens_pair(outps_p, g):
                        # den row: psum partition 64 -> sbuf partition 0
                        # (cross-quadrant nch=1 move; the custom recip
                        # misbehaves at base partition 64), then approx recip
                        recs = []
                        for j in range(2):
                            den = pb.tile([1, 512], fp32, name="den",
                                          tag="den", bufs=8)
                            nc.vector.tensor_copy(
                                den[0:1, :], outps_p[g][DK:DK + 1, j, :]
                            )
                            rec = pb.tile([1, 512], mdt, name="rec",
                                          tag="rec", bufs=8)
                            nc.vector._custom_dve(
                                _RF, out=rec[0:1, :], in0=den[0:1, :],
                                s0=_RC["s0"], s1=_RC["s1"], imm2=_RC["imm2"],
                            )
                            recs.append(rec)
                        return recs

                    def emit_bc(outps_p, recs, hh):
                        # broadcast 1/den over the attn rows of the outps
                        # bank (values already saved to ocp) -- base 0, no
                        # PE tiling involved
                        g, j = hh // 2, hh % 2
                        nc.tensor.matmul(
                            outps_p[g][0:DK, j, :], ones[0:1, 0:DK],
                            recs[hh][0:1, :], start=True, stop=True,
                        )

                    def emit_finish(outps_p, ocps, p4_p):
                        # oTP = bc broadcast (psum) * saved attn (sbuf)
                        for hh in range(4):
                            g, j = hh // 2, hh % 2
                            h = 4 * p4_p + 2 * g + j
                            with nc.allow_low_precision(
                                reason="bf16 output-projection operands"
                            ):
                                nc.vector.tensor_mul(
                                    oTP[DK * j:DK * j + DK, h // 2, :],
                                    outps_p[g][0:DK, j, :],
                                    ocps[g][:, j, :],
                                )

                    prev = None  # (outps, p4, recs) of the previous pass
                    for p4 in range(4):
                        # outps[g]: [128, 2(j), 512]; attnV accumulates into
                        # rows 0:65 (64 dv + den), rows 64:128 later receive
                        # the bc broadcast of the reciprocal
                        outps = [
                            pbps.tile([P, 2, 512], fp32,
                                      name=f"outps{g}", tag=f"outps{g}",
                                      bufs=1)
                            for g in range(2)
                        ]
                        exs = {}

                        def emit_attnv(tb):
                            for g in range(2):
                                for j in range(2):
                                    nc.tensor.matmul(
                                        outps[g][0:DK + 1, j, :], Vp[:, tb, :],
                                        exs[(tb, g)][:, j, :],
                                        start=(tb == 0), stop=(tb == NT - 1),
                                    )

                        for tb in range(NT):
                            for g in range(2):
                                sc = pbps.tile([P, 2, 512], fp32, name=f"sc{g}",
                                               tag=f"sc{g}", bufs=1)
                                for j in range(2):
                                    h = 4 * p4 + 2 * g + j
                                    r = (h % 2) * DK
                                    nc.tensor.matmul(
                                        sc[:, j, :],
                                        K2T[r:r + DK, ts(tb, P)],
                                        QT[r:r + DK, h // 2, :],
                                        start=True, stop=True,
                                    )
                                ex = pb.tile([P, 2, 512], mdt, name=f"ex{g}",
                                             tag=f"ex{g}", bufs=7)
                                nc.scalar.activation(ex[:], sc[:], Act.Exp, scale=scale)
                                exs[(tb, g)] = ex
                            if prev is not None and tb == 1:
                                emit_bc(prev[0], prev[3], 0)
                                emit_bc(prev[0], prev[3], 1)
                            if prev is not None and tb == 4:
                                emit_bc(prev[0], prev[3], 2)
                                emit_bc(prev[0], prev[3], 3)
                                emit_finish(prev[0], prev[2], prev[1])
                            for k in SCHED[tb]:
                                emit_attnv(k)
                        for k in TAIL_ATTNV:
                            emit_attnv(k)
                        last = (p4 == 3)
                        ocps, recs = [], []
                        for g in range(2):
                            ocps += emit_ocp_one(outps, g, on_act=last)
                            recs += emit_dens_pair(outps, g)
                        prev = (outps, p4, ocps, recs)

                    # last pass: normalize at the tail, with phase C's first
                    # matmul groups interleaved to keep the PE warm

                    # ---------------- Phase C: output projection ----------
                    # ypair[sb]: [128, 2(df), 512] accumulators in the banks
                    # the sc tiles just released; heads of passes 0..2
                    # (g8 0..5) accumulate during the last normalize chain
                    def emit_cmm(ypair, sb, df, g8s, start, stop):
                        for g8 in g8s:
                            nc.tensor.matmul(
                                ypair[:, df, :],
                                oTP[:, g8, ts(sb, P)],
                                wo_sb[:, g8, ts(df, 512)],
                                start=(start and g8 == g8s[0]),
                                stop=(stop and g8 == g8s[-1]),
                            )

                    # groups 0..5 (heads of passes 0-2) for ALL sb blocks
                    # run during the last pass's normalize chain; groups 6,7
                    # complete each block as soon as the muls land
                    ypairs = {}
                    tags = ["sc0", "sc1", "outps0", "outps1"]
                    for sb in range(NSB):
                        ypairs[sb] = pbps.tile([P, 2, 512], fp32,
                                               name=f"yp{sb}", tag=tags[sb],
                                               bufs=1)
                        emit_cmm(ypairs[sb], sb, 0, list(range(6)), True, False)
                        emit_cmm(ypairs[sb], sb, 1, list(range(6)), True, False)
                        if sb == 0:
                            emit_bc(prev[0], prev[3], 0)
                            emit_bc(prev[0], prev[3], 1)
                        elif sb == 1:
                            emit_bc(prev[0], prev[3], 2)
                            emit_bc(prev[0], prev[3], 3)
                        elif sb == 2:
                            emit_finish(prev[0], prev[2], prev[1])

                    for sb in range(NSB):
                        yp = ypairs[sb]
                        for df in range(NDF):
                            emit_cmm(yp, sb, df, [6, 7], False, True)
                            ysb = pb.tile([P, 512], fp32, name="ysb",
                                          tag="ysb", bufs=3)
                            nc.vector.tensor_copy(ysb[:], yp[:, df, :])
                            eng = (nc.sync, nc.scalar, nc.gpsimd)[(2 * sb + df) % 3]
                            eng.dma_start(y[ts(sb, P), ts(df, 512)], ysb[:])

    nc.compile()
    return nc


def make_in_maps(x, w_q, w_k, w_v, w_out):
    cst = np.zeros((P, P), dtype=np.float32)
    cst[0:DK, 0:DK] = np.eye(DK, dtype=np.float32)
    cst[:, DK:] = 1.0
    x = np.ascontiguousarray(np.asarray(x, dtype=np.float32))
    w_q = np.asarray(w_q, dtype=np.float32)
    w_k = np.asarray(w_k, dtype=np.float32)
    w_v = np.asarray(w_v, dtype=np.float32)
    w_out = np.asarray(w_out, dtype=np.float32)

    import ml_dtypes
    bf = ml_dtypes.bfloat16
    # pre-tile every operand into the exact SBUF layout the kernel loads, so
    # each DMA is a contiguous slab: [pi, (c,) po, cols]. The projection
    # inputs are bf16: halves DMA and enables fast weight loads; softmax
    # renormalization cancels most of the scores error.
    wqH = np.ascontiguousarray(
        w_q.T.reshape(KC, P, D).transpose(1, 0, 2)).astype(bf)
    wkkT = np.concatenate([w_k.T, w_k.T], axis=1)  # [D, 128]
    wkkH = np.ascontiguousarray(
        wkkT.reshape(KC, P, P).transpose(1, 0, 2)).astype(bf)
    wvH = np.ascontiguousarray(
        w_v.T.reshape(KC, P, DK).transpose(1, 0, 2)).astype(bf)
    # woP[64*(h%2)+dv, h//2, :] = w_out.T[h*DK+dv, :], in bf16 (the output
    # projection runs bf16 x bf16 -> fp32)
    woP = np.ascontiguousarray(
        w_out.T.reshape(KC, 2, DK, D).transpose(1, 2, 0, 3).reshape(P, KC, D)
    ).astype(bf)

    in_maps = []
    for c in range(NCORES):
        b, r = divmod(c, GPB)
        # roll this core's query rows to the front; t-order is irrelevant
        # (attention sums over t), so K/V are unaffected
        xb = np.roll(x[b], -r * SPB, axis=0)
        # xS[pi, c, po, s] so each c-block is one contiguous slab/partition
        xS = np.ascontiguousarray(
            xb.T.reshape(KC, P, 4, 512).transpose(1, 2, 0, 3)
        ).astype(bf)
        in_maps.append(
            {"xS": xS, "wqH": wqH, "wkkH": wkkH, "wvH": wvH, "woP": woP,
             "cst": cst}
        )
    return in_maps


_BUILD_CACHE = {}


def _cached_nc(scale: float):
    key = round(float(scale), 12)
    if key not in _BUILD_CACHE:
        _BUILD_CACHE[key] = build_bass(float(scale))
    return _BUILD_CACHE[key]


def run_on_hw(in_maps, scale, trace=False):
    from concourse.bass_utils import run_bass_kernel_spmd

    nc = _cached_nc(scale)
    return run_bass_kernel_spmd(nc, in_maps, list(range(NCORES)), trace=trace)


def assemble(results):
    out = np.empty((B, S, D), dtype=np.float32)
    for c in range(NCORES):
        b, r = divmod(c, GPB)
        out[b, r * SPB:(r + 1) * SPB] = results[c]["y"]
    return out


def kernel(x, w_q, w_k, w_v, w_out, softmax_scale):
    scale = float(np.asarray(softmax_scale).reshape(-1)[0])
    in_maps = make_in_maps(x, w_q, w_k, w_v, w_out)
    res = run_on_hw(in_maps, scale, trace=False)
    return assemble(res.results)



# revision 27
# speedup vs baseline: 1.0188x; 1.0188x over previous
"""Multi-head attention (multiquery K/V) Bass kernel for 8 trn2 NeuronCores.

Sharding: 8 cores = 2 batches x 4 query-row quarters. Each core computes the
full multiquery K/V projections for its batch (cheap, dk=64) and runs
attention + output projection for its 512 query rows over all 16 heads.
Output is a pure concatenation across cores -- no collectives.

Device data flow (everything "transposed" so matmuls contract on partitions):
  xS (host pre-tiled, bf16): contiguous per-partition slabs, split across
     the sync/scalar/gpsimd DMA queues (one queue tops out ~120GB/s)
  QT [dout, s]  = x @ w_q.T transposed (bf16 inputs -> fp32 psum -> f32r)
  K2T [128, t]  = K.T stacked twice (rows 0:64 and 64:128) so even/odd heads
                  run as concurrent row-tiled matmul pairs
  Vp [t, 65]    = V with a ones column (column 64 accumulates the softmax
                  denominator during the attn@V matmul)
  phase B per 4-head pass: scores_T[t,s] -> exp on ACT (the ~147us kernel
  floor; scale folded in) -> attn@V, software-pipelined with a tapered
  depth-5 schedule. The previous pass's normalize (save attn values to
  SBUF, approx-reciprocal of the denominator row, rank-1 matmul broadcast
  into the freed psum rows, multiply) is interleaved into the next pass's
  loop so ACT never idles.
  phase C: y = sum_g8 oTP[:,g8] @ woP[:,g8] (head pairs packed to K=128,
  bf16) with groups 0..5 emitted during the last normalize chain.
"""

import sys

import numpy as np

if "/opt/trn_rl_repo" not in sys.path:
    sys.path.insert(0, "/opt/trn_rl_repo")

B, S, D = 2, 2048, 1024
H, DK = 16, 64
P = 128
NCORES, GPB = 8, 4
SPB = S // GPB  # 512 query rows per core
KC = D // P  # 8 contraction subtiles over d_model
NT = S // P  # 16 key/t blocks
NSB = SPB // P  # 4 s blocks
NDF = D // 512  # 2 output column halves


USE_F32R = True


def build_bass(scale: float):
    import concourse.bacc as bacc
    import concourse.mybir as mybir
    import concourse.tile as tile
    from concourse.bass import ts

    fp32 = mybir.dt.float32
    bf16 = mybir.dt.bfloat16
    # matmul-operand dtype: float32r streams at 1 cycle/row (vs 4 for
    # float32) and is bit-identical fp32 in memory
    mdt = mybir.dt.float32r if USE_F32R else fp32
    Act = mybir.ActivationFunctionType

    nc = bacc.Bacc(None, target_bir_lowering=False)
    # host-pre-tiled layouts: every DMA below is a contiguous slab per
    # partition (no software descriptor generation on gpsimd)
    xS = nc.dram_tensor("xS", [P, 4 * KC, 512], bf16, kind="ExternalInput")
    cst = nc.dram_tensor("cst", [P, P], mdt, kind="ExternalInput")
    wqH = nc.dram_tensor("wqH", [P, KC, D], bf16, kind="ExternalInput")
    wkkH = nc.dram_tensor("wkkH", [P, KC, P], bf16, kind="ExternalInput")
    wvH = nc.dram_tensor("wvH", [P, KC, DK], bf16, kind="ExternalInput")
    woP = nc.dram_tensor("woP", [P, KC, D], bf16, kind="ExternalInput")
    y = nc.dram_tensor("y", [SPB, D], fp32, kind="ExternalOutput")

    with tile.TileContext(nc) as tc:
        with tc.tile_pool(name="persist", bufs=1) as pers:
            K2T = pers.tile([P, S], mdt, name="K2T")
            Vp = pers.tile([P, NT, DK + 1], mdt, name="Vp")
            QT = pers.tile([P, KC, SPB], mdt, name="QT")
            oTP = pers.tile([P, KC, SPB], bf16, name="oTP")
            cst_sb = pers.tile([P, P], mdt, name="cst")
            # cst cols 0:64 = identity(64) (rows 0:64), cols 64:128 = ones
            nc.sync.dma_start(cst_sb[:], cst[:])
            ident = cst_sb[0:DK, 0:DK]
            ones = cst_sb[:, DK:DK + DK]

            # ---------------- Phase A: projections ----------------
            with (
                tc.tile_pool(name="pa", bufs=1) as pa,
                tc.tile_pool(name="paps", bufs=1, space="PSUM") as paps,
            ):
                wkk_sb = pa.tile([P, KC, P], bf16, name="wkk")
                wv_sb = pa.tile([P, KC, DK], bf16, name="wv")
                nc.sync.dma_start(wkk_sb[:], wkkH[:])
                nc.gpsimd.dma_start(wv_sb[:], wvH[:])
                wq_sb = pa.tile([P, KC, D], bf16, name="wq")

                for c in range(4):
                    xc = pa.tile([P, KC, 512], bf16, name="xc", tag="xc", bufs=2)
                    # x blocks spread across the three DMA queues (a single
                    # queue tops out well below the per-core HBM bandwidth);
                    # xc1/xc3 ride the gpsimd queue, which carries nothing
                    # else after startup
                    o = c * KC
                    if c == 0:
                        nc.sync.dma_start(xc[:, 0:3, :], xS[:, o:o + 3, :])
                        nc.scalar.dma_start(xc[:, 3:6, :], xS[:, o + 3:o + 6, :])
                        nc.gpsimd.dma_start(xc[:, 6:8, :], xS[:, o + 6:o + 8, :])
                        # ones column of Vp (strided; keep off the xc0 path)
                        nc.gpsimd.dma_start(Vp[:, :, DK], cst[:, DK:DK + NT])
                        # w_q after xc0 so the first K/V matmuls and the Q
                        # k-loop aren't stuck behind the 4MB transfer
                        for k in range(KC):
                            (nc.sync if k % 2 == 0 else nc.scalar).dma_start(
                                wq_sb[:, k, :], wqH[:, k, :]
                            )
                    elif c == 1 or c == 3:
                        nc.gpsimd.dma_start(xc[:], xS[:, o:o + KC, :])
                    else:
                        nc.sync.dma_start(xc[:, 0:4, :], xS[:, o:o + 4, :])
                        nc.scalar.dma_start(xc[:, 4:8, :], xS[:, o + 4:o + 8, :])

                    k2ps = paps.tile([P, 512], fp32, name="k2ps", tag="k2ps", bufs=2)
                    for k in range(KC):
                        nc.tensor.matmul(
                            k2ps[:], wkk_sb[:, k, :], xc[:, k, :],
                            start=(k == 0), stop=(k == KC - 1),
                        )
                    nc.vector.tensor_copy(K2T[:, ts(c, 512)], k2ps[:])

                    # V.T then PE-transpose into V' natural [t, dv]
                    vps = paps.tile([DK, 512], fp32, name="vps", tag="vps", bufs=2)
                    for k in range(KC):
                        nc.tensor.matmul(
                            vps[:], wv_sb[:, k, :], xc[:, k, :],
                            start=(k == 0), stop=(k == KC - 1),
                        )
                    vsb = pa.tile([DK, 512], mdt, name="vsb", tag="vsb", bufs=2)
                    nc.vector.tensor_copy(vsb[:], vps[:])
                    for q in range(4):
                        trps = paps.tile([P, DK], mdt, name="trps", tag="trps", bufs=2)
                        nc.tensor.transpose(trps[:], vsb[:, ts(q, P)], ident)
                        nc.vector.tensor_copy(Vp[:, 4 * c + q, 0:DK], trps[:])

                    if c == 0:
                        for m in range(KC):
                            qps = paps.tile([P, 512], fp32, name="qps", tag="qps", bufs=2)
                            for k in range(KC):
                                nc.tensor.matmul(
                                    qps[:], wq_sb[:, k, ts(m, P)], xc[:, k, :],
                                    start=(k == 0), stop=(k == KC - 1),
                                )
                            nc.vector.tensor_copy(QT[:, m, :], qps[:])

            # ---------------- Phase B: attention ----------------
            with tc.tile_pool(name="pb", bufs=1) as pb:
                wo_sb = pb.tile([P, KC, D], bf16, name="wo")
                nc.sync.dma_start(wo_sb[:], woP[:])

                from concourse.dve_ops import (
                    RECIP_APPROX_FAST_CONSTS as _RC,
                    RECIPROCAL_APPROX_FAST as _RF,
                )

                with tc.tile_pool(name="pbps", bufs=1, space="PSUM") as pbps:
                    # attnV emission schedule: deep (5) at pass start so the
                    # previous pass's normalize has time to release the outps
                    # banks, shallow at pass end so the PE tail is short and
                    # the next pass's scores/exp restart quickly
                    SCHED = {t: [] for t in range(NT)}
                    for k in range(9):
                        SCHED[k + 5].append(k)
                    SCHED[14] += [9, 10]
                    SCHED[15] += [11, 12, 13]
                    TAIL_ATTNV = [14, 15]

                    def emit_ocp_one(outps_p, g, on_act=False):
                        # save attn values to SBUF; frees the outps bank to
                        # receive the bc broadcast. On the last pass ACT is
                        # idle, so the copies go there.
                        ocp = pb.tile([DK, 2, 512], fp32,
                                      name=f"ocp{g}", tag=f"ocp{g}", bufs=2)
                        if on_act:
                            nc.scalar.copy(ocp[:], outps_p[g][0:DK, :, :])
                        else:
                            nc.vector.tensor_copy(
                                ocp[:], outps_p[g][0:DK, :, :]
                            )
                        return [ocp]

                    def emit_dens_pair(outps_p, g):
                        # den row: psum partition 64 -> sbuf partition 0
                        # (cross-quadrant nch=1 move; the custom recip
                        # misbehaves at base partition 64), then approx recip
                        recs = []
                        for j in range(2):
                            den = pb.tile([1, 512], fp32, name="den",
                                          tag="den", bufs=8)
                            nc.vector.tensor_copy(
                                den[0:1, :], outps_p[g][DK:DK + 1, j, :]
                            )
                            rec = pb.tile([1, 512], mdt, name="rec",
                                          tag="rec", bufs=8)
                            nc.vector._custom_dve(
                                _RF, out=rec[0:1, :], in0=den[0:1, :],
                                s0=_RC["s0"], s1=_RC["s1"], imm2=_RC["imm2"],
                            )
                            recs.append(rec)
                        return recs

                    def emit_bc(outps_p, recs, hh):
                        # broadcast 1/den over the attn rows of the outps
                        # bank (values already saved to ocp) -- base 0, no
                        # PE tiling involved
                        g, j = hh // 2, hh % 2
                        nc.tensor.matmul(
                            outps_p[g][0:DK, j, :], ones[0:1, 0:DK],
                            recs[hh][0:1, :], start=True, stop=True,
                        )

                    def emit_finish(outps_p, ocps, p4_p):
                        # oTP = bc broadcast (psum) * saved attn (sbuf)
                        for hh in range(4):
                            g, j = hh // 2, hh % 2
                            h = 4 * p4_p + 2 * g + j
                            with nc.allow_low_precision(
                                reason="bf16 output-projection operands"
                            ):
                                nc.vector.tensor_mul(
                                    oTP[DK * j:DK * j + DK, h // 2, :],
                                    outps_p[g][0:DK, j, :],
                                    ocps[g][:, j, :],
                                )

                    prev = None  # (outps, p4, recs) of the previous pass
                    for p4 in range(4):
                        # outps[g]: [128, 2(j), 512]; attnV accumulates into
                        # rows 0:65 (64 dv + den), rows 64:128 later receive
                        # the bc broadcast of the reciprocal
                        outps = [
                            pbps.tile([P, 2, 512], fp32,
                                      name=f"outps{g}", tag=f"outps{g}",
                                      bufs=1)
                            for g in range(2)
                        ]
                        exs = {}

                        def emit_attnv(tb):
                            for g in range(2):
                                for j in range(2):
                                    nc.tensor.matmul(
                                        outps[g][0:DK + 1, j, :], Vp[:, tb, :],
                                        exs[(tb, g)][:, j, :],
                                        start=(tb == 0), stop=(tb == NT - 1),
                                    )

                        for tb in range(NT):
                            for g in range(2):
                                sc = pbps.tile([P, 2, 512], fp32, name=f"sc{g}",
                                               tag=f"sc{g}", bufs=1)
                                for j in range(2):
                                    h = 4 * p4 + 2 * g + j
                                    r = (h % 2) * DK
                                    nc.tensor.matmul(
                                        sc[:, j, :],
                                        K2T[r:r + DK, ts(tb, P)],
                                        QT[r:r + DK, h // 2, :],
                                        start=True, stop=True,
                                    )
                                ex = pb.tile([P, 2, 512], mdt, name=f"ex{g}",
                                             tag=f"ex{g}", bufs=7)
                                nc.scalar.activation(ex[:], sc[:], Act.Exp, scale=scale)
                                exs[(tb, g)] = ex
                            if prev is not None and tb == 1:
                                emit_bc(prev[0], prev[3], 0)
                                emit_bc(prev[0], prev[3], 1)
                            if prev is not None and tb == 4:
                                emit_bc(prev[0], prev[3], 2)
                                emit_bc(prev[0], prev[3], 3)
                                emit_finish(prev[0], prev[2], prev[1])
                            for k in SCHED[tb]:
                                emit_attnv(k)
                        for k in TAIL_ATTNV:
                            emit_attnv(k)
                        last = (p4 == 3)
                        ocps, recs = [], []
                        for g in range(2):
                            ocps += emit_ocp_one(outps, g, on_act=last)
                            recs += emit_dens_pair(outps, g)
                        prev = (outps, p4, ocps, recs)

                    # last pass: normalize at the tail, with phase C's first
                    # matmul groups interleaved to keep the PE warm

                    # ---------------- Phase C: output projection ----------
                    # ypair[sb]: [128, 2(df), 512] accumulators in the banks
                    # the sc tiles just released; heads of passes 0..2
                    # (g8 0..5) accumulate during the last normalize chain
                    def emit_cmm(ypair, sb, df, g8s, start, stop):
                        for g8 in g8s:
                            nc.tensor.matmul(
                                ypair[:, df, :],
                                oTP[:, g8, ts(sb, P)],
                                wo_sb[:, g8, ts(df, 512)],
                                start=(start and g8 == g8s[0]),
                                stop=(stop and g8 == g8s[-1]),
                            )

                    ypairs = {}
                    for sb in range(2):
                        ypairs[sb] = pbps.tile([P, 2, 512], fp32,
                                               name=f"yp{sb}", tag=f"sc{sb}",
                                               bufs=1)
                        emit_cmm(ypairs[sb], sb, 0, list(range(6)), True, False)
                        emit_cmm(ypairs[sb], sb, 1, list(range(6)), True, False)
                        if sb == 0:
                            emit_bc(prev[0], prev[3], 0)
                            emit_bc(prev[0], prev[3], 1)
                        else:
                            emit_bc(prev[0], prev[3], 2)
                            emit_bc(prev[0], prev[3], 3)
                    emit_finish(prev[0], prev[2], prev[1])

                    for sb in range(NSB):
                        if sb not in ypairs:
                            ypairs[sb] = pbps.tile([P, 2, 512], fp32,
                                                   name=f"yp{sb}",
                                                   tag=f"sc{sb % 2}", bufs=1)
                        yp = ypairs[sb]
                        for df in range(NDF):
                            if sb in (0, 1):
                                emit_cmm(yp, sb, df, [6, 7], False, True)
                            else:
                                emit_cmm(yp, sb, df, list(range(KC)),
                                         True, True)
                            ysb = pb.tile([P, 512], fp32, name="ysb",
                                          tag="ysb", bufs=3)
                            nc.vector.tensor_copy(ysb[:], yp[:, df, :])
                            eng = (nc.sync, nc.scalar, nc.gpsimd)[(2 * sb + df) % 3]
                            eng.dma_start(y[ts(sb, P), ts(df, 512)], ysb[:])

    nc.compile()
    return nc


def make_in_maps(x, w_q, w_k, w_v, w_out):
    cst = np.zeros((P, P), dtype=np.float32)
    cst[0:DK, 0:DK] = np.eye(DK, dtype=np.float32)
    cst[:, DK:] = 1.0
    x = np.ascontiguousarray(np.asarray(x, dtype=np.float32))
    w_q = np.asarray(w_q, dtype=np.float32)
    w_k = np.asarray(w_k, dtype=np.float32)
    w_v = np.asarray(w_v, dtype=np.float32)
    w_out = np.asarray(w_out, dtype=np.float32)

    import ml_dtypes
    bf = ml_dtypes.bfloat16
    # pre-tile every operand into the exact SBUF layout the kernel loads, so
    # each DMA is a contiguous slab: [pi, (c,) po, cols]. The projection
    # inputs are bf16: halves DMA and enables fast weight loads; softmax
    # renormalization cancels most of the scores error.
    wqH = np.ascontiguousarray(
        w_q.T.reshape(KC, P, D).transpose(1, 0, 2)).astype(bf)
    wkkT = np.concatenate([w_k.T, w_k.T], axis=1)  # [D, 128]
    wkkH = np.ascontiguousarray(
        wkkT.reshape(KC, P, P).transpose(1, 0, 2)).astype(bf)
    wvH = np.ascontiguousarray(
        w_v.T.reshape(KC, P, DK).transpose(1, 0, 2)).astype(bf)
    # woP[64*(h%2)+dv, h//2, :] = w_out.T[h*DK+dv, :], in bf16 (the output
    # projection runs bf16 x bf16 -> fp32)
    woP = np.ascontiguousarray(
        w_out.T.reshape(KC, 2, DK, D).transpose(1, 2, 0, 3).reshape(P, KC, D)
    ).astype(bf)

    in_maps = []
    for c in range(NCORES):
        b, r = divmod(c, GPB)
        # roll this core's query rows to the front; t-order is irrelevant
        # (attention sums over t), so K/V are unaffected
        xb = np.roll(x[b], -r * SPB, axis=0)
        # xS[pi, c, po, s] so each c-block is one contiguous slab/partition
        xS = np.ascontiguousarray(
            xb.T.reshape(KC, P, 4, 512).transpose(1, 2, 0, 3)
        ).astype(bf)
        in_maps.append(
            {"xS": xS, "wqH": wqH, "wkkH": wkkH, "wvH": wvH, "woP": woP,
             "cst": cst}
        )
    return in_maps


_BUILD_CACHE = {}


def _cached_nc(scale: float):
    key = round(float(scale), 12)
    if key not in _BUILD_CACHE:
        _BUILD_CACHE[key] = build_bass(float(scale))
    return _BUILD_CACHE[key]


def run_on_hw(in_maps, scale, trace=False):
    from concourse.bass_utils import run_bass_kernel_spmd

    nc = _cached_nc(scale)
    return run_bass_kernel_spmd(nc, in_maps, list(range(NCORES)), trace=trace)


def assemble(results):
    out = np.empty((B, S, D), dtype=np.float32)
    for c in range(NCORES):
        b, r = divmod(c, GPB)
        out[b, r * SPB:(r + 1) * SPB] = results[c]["y"]
    return out


def kernel(x, w_q, w_k, w_v, w_out, softmax_scale):
    scale = float(np.asarray(softmax_scale).reshape(-1)[0])
    in_maps = make_in_maps(x, w_q, w_k, w_v, w_out)
    res = run_on_hw(in_maps, scale, trace=False)
    return assemble(res.results)

